# revision 1
# baseline (speedup 1.0000x reference)
"""ConvDeepSet SPMD kernel for 8 Trainium2 NeuronCores.

Math (per batch b, all fp32 in reference):
    density = 1 where wt[:,0] finite else 0            [1,W,H]
    wt_aug  = concat([density, nan_to_num(wt)])        [CC=33,W,H]
    w0[w,x] = exp(-0.5*(lon_in[w]-lon_out[x])^2/ls^2)  [W,X]
    w1[h,y] = exp(-0.5*(lat_in[h]-lat_out[y])^2/ls^2)  [H,Y]
    ee[c,x,y] = sum_{w,h} wt_aug[c,w,h]*w0[w,x]*w1[h,y]
    out[0]   = ee[0];  out[c>=1] = ee[c] / clip(ee[0], 1e-6, 1e5)

Sharding: data-parallel over batch B=8 -> one NeuronCore per batch.
Per-core compute is two chained matmuls per channel:
    stage1: T1[h, x] = wtr[:, c*H:(c+1)*H].T @ w0      (contract W=256, 2 K-tiles)
    stage2: ee[x, y] = T1[:, xs].T @ w1                (contract H=128)
plus the RBF weights built on-device (ACT Square/Exp with per-partition
bias), a clipped-reciprocal density normalization on DVE, and outputs
staged per (channel-group, x-stripe) for batched dual-queue HWDGE DMA.

Pipelining: channels are processed in units (singles or pairs); each
unit's stage-1 is emitted BEFORE the previous unit's stage-2 so the PE
never stalls on the ACT psum->sbuf T1 copy. Channel pairs share one
2-bank PSUM tile so a single strided DVE op scales both by the density
reciprocal.
"""

import sys
from contextlib import ExitStack

import numpy as np

sys.path.insert(0, "/opt/trn_rl_repo")

import concourse.bass as bass  # noqa: E402,F401
import concourse.tile as tile  # noqa: E402
from concourse import bacc, mybir  # noqa: E402
from concourse.bass_utils import run_bass_kernel_spmd  # noqa: E402

B, C, W, H, X, Y = 8, 32, 256, 128, 720, 361
CC = C + 1          # channels incl. density
KT = W // 128       # stage-1 K tiles (2)
N1 = 360            # stage-1 moving split (720 = 2x360, <=512 per PSUM bank)
XOFF = [0, 128, 256, 384, 512, 640]   # stage-2 x stripes (5x128 + 80)
XLEN = [128, 128, 128, 128, 128, 80]
NXT = len(XOFF)
CG = 4              # output channels batched per DMA group

F32 = mybir.dt.float32

# matmul precision for the two big stages: "f32" | "f32r" | "bf16"
MM_DTYPE = "bf16"
TRACE = False
LAST_RESULT = None

_cache = {}


def _units():
    """Channel processing units: singles or in-group pairs (CG=8).

    Channel 0 (density) is alone; pairs never cross a CG output group
    because a pair's two stripes land in one staging tile.
    """
    units = [[0], [1, 2], [3]]
    for g in range(1, 8):
        units.append([4 * g, 4 * g + 1])
        units.append([4 * g + 2, 4 * g + 3])
    units.append([32])
    return units


def _build(alpha: float, mm: str):
    nc = bacc.Bacc(
        "TRN2",
        target_bir_lowering=False,
        debug=False,
        enable_asserts=False,
        num_devices=B,
    )
    mmdt = {"f32": F32, "f32r": mybir.dt.float32r, "bf16": mybir.dt.bfloat16}[mm]

    wtr = nc.dram_tensor("wtr", [W, CC * H], mmdt, kind="ExternalInput").ap()
    lon_in = nc.dram_tensor("lon_in", [1, W], F32, kind="ExternalInput").ap()
    lon_out = nc.dram_tensor("lon_out", [1, X], F32, kind="ExternalInput").ap()
    lat_in = nc.dram_tensor("lat_in", [1, H], F32, kind="ExternalInput").ap()
    lat_out = nc.dram_tensor("lat_out", [1, Y], F32, kind="ExternalInput").ap()
    out = nc.dram_tensor("out", [CC, X, Y], F32, kind="ExternalOutput").ap()

    with tile.TileContext(nc) as tc, ExitStack() as ctx:
        wtr_pool = ctx.enter_context(tc.tile_pool(name="wtr", bufs=KT))
        w0_pool = ctx.enter_context(tc.tile_pool(name="w0", bufs=KT))
        w1_pool = ctx.enter_context(tc.tile_pool(name="w1", bufs=1))
        t1sb_pool = ctx.enter_context(tc.tile_pool(name="t1sb", bufs=6))
        recip_pool = ctx.enter_context(tc.tile_pool(name="recip", bufs=2 * NXT))
        outsb_pool = ctx.enter_context(tc.tile_pool(name="outsb", bufs=18))
        small_pool = ctx.enter_context(tc.tile_pool(name="small", bufs=2))
        t1ps_pool = ctx.enter_context(tc.tile_pool(name="t1ps", bufs=2, space="PSUM"))
        eeps_pool = ctx.enter_context(tc.tile_pool(name="eeps", bufs=3, space="PSUM"))

        # ---- load wt (stage-1 stationary operand), [W, CC*H] split in K tiles.
        # Chunked by channel block and spread over both HWDGE queues so the
        # first stage-1 matmuls start after ~1/4 of the load.
        wtr_sb = []
        chunks = [(0, 9), (9, 17), (17, 25), (25, 33)]
        for k in range(KT):
            t = wtr_pool.tile([128, CC * H], mmdt, tag="wtr", name=f"wtr_sb{k}")
            wtr_sb.append(t)
        # ---- RBF weights: w[p, x] = exp(alpha * (a_p - b_x)^2) on ACT,
        # with b broadcast across partitions and a as per-partition bias.
        def rbf(in_ap, out_ap, n_in, n_out, w_sb):
            bb = small_pool.tile([128, n_out], F32, tag="rbf_bb", name=f"rbf_bb{n_out}")
            nc.sync.dma_start(bb[:], out_ap.to_broadcast([128, n_out]))
            for k in range(n_in // 128):
                ar = small_pool.tile(
                    [128, 1], F32, tag="rbf_ar", name=f"rbf_ar{n_in}_{k}"
                )
                nc.sync.dma_start(
                    ar[:],
                    in_ap[0:1, k * 128 : (k + 1) * 128].rearrange("a b -> b a"),
                )
                d2 = small_pool.tile(
                    [128, n_out], F32, tag="rbf_d2", name=f"rbf_d2{n_in}_{k}"
                )
                # d2 = (a - b)^2 = Square(bb * -1 + a)
                nc.scalar.activation(
                    d2[:],
                    bb[:],
                    mybir.ActivationFunctionType.Square,
                    bias=ar[:],
                    scale=-1.0,
                )
                # w = exp(alpha * d2), alpha = -0.5/ls^2
                nc.scalar.activation(
                    w_sb[k][:],
                    d2[:],
                    mybir.ActivationFunctionType.Exp,
                    scale=alpha,
                )

        w0_sb = [
            w0_pool.tile([128, X], mmdt, tag="w0", name=f"w0_sb{k}")
            for k in range(KT)
        ]
        rbf(lon_in, lon_out, W, X, w0_sb)
        w1_sb = [w1_pool.tile([128, Y], mmdt, tag="w1", name="w1_sb0")]
        rbf(lat_in, lat_out, H, Y, w1_sb)
        w1_sb = w1_sb[0]

        for ci, (a, b) in enumerate(chunks):
            for k in range(KT):
                eng = nc.sync if (ci + k) % 2 == 0 else nc.scalar
                eng.dma_start(
                    wtr_sb[k][:, a * H : b * H],
                    wtr[k * 128 : (k + 1) * 128, a * H : b * H],
                )

        # ---- stage 1 for one channel: T1[h, x] psum (two 1-bank halves),
        # ACT-copied (and rounded to the matmul dtype) into SBUF.
        def stage1(c):
            t1sb = t1sb_pool.tile([128, X], mmdt, tag="t1sb", name=f"t1sb_c{c}")
            for n in range(2):
                t1ps = t1ps_pool.tile(
                    [128, N1], F32, tag="t1ps", name=f"t1ps_c{c}_{n}"
                )
                for k in range(KT):
                    nc.tensor.matmul(
                        t1ps[:],
                        wtr_sb[k][:, c * H : (c + 1) * H],
                        w0_sb[k][:, n * N1 : (n + 1) * N1],
                        start=(k == 0),
                        stop=(k == KT - 1),
                    )
                nc.scalar.copy(t1sb[:, n * N1 : (n + 1) * N1], t1ps[:])
            return t1sb

        units = _units()
        recips = []
        stage_tiles = [None] * NXT

        def emit_stage2(unit, t1sbs):
            c0 = unit[0]
            g = c0 // CG
            gc = min(CG, CC - g * CG)
            ci0 = c0 % CG
            for j in range(NXT):
                xo, xl = XOFF[j], XLEN[j]
                eep = eeps_pool.tile(
                    [128, 1024], F32, tag="ee", name=f"ee_u{c0}_{j}"
                )
                for idx in range(len(unit)):
                    nc.tensor.matmul(
                        eep[0:xl, idx * 512 : idx * 512 + Y],
                        t1sbs[idx][:, xo : xo + xl],
                        w1_sb[:],
                        start=True,
                        stop=True,
                    )
                if ci0 == 0:
                    stage_tiles[j] = outsb_pool.tile(
                        [128, CG * Y], F32, tag="stage", name=f"stage_g{g}_{j}"
                    )
                st = stage_tiles[j]
                if c0 == 0:
                    # density channel: copy out (ACT) + clipped reciprocal (DVE)
                    nc.scalar.copy(st[0:xl, 0:Y], eep[0:xl, 0:Y])
                    r = recip_pool.tile([128, Y], F32, tag="recip", name=f"recip{j}")
                    rs = recip_pool.tile(
                        [128, Y], F32, tag="recip", name=f"recip_s{j}"
                    )
                    nc.vector.tensor_scalar(
                        rs[0:xl, :],
                        eep[0:xl, 0:Y],
                        1e-6,
                        1e5,
                        mybir.AluOpType.max,
                        mybir.AluOpType.min,
                    )
                    nc.vector.reciprocal_approx_fast(r[0:xl, :], rs[0:xl, :])
                    recips.append(r)
                elif len(unit) == 2:
                    # scale both channels with one strided DVE op
                    src2 = eep[0:xl, :].rearrange("p (b y) -> p b y", b=2)[:, :, 0:Y]
                    dst = st[0:xl, ci0 * Y : (ci0 + 2) * Y].rearrange(
                        "p (b y) -> p b y", b=2
                    )
                    rr = recips[j][0:xl, :].unsqueeze(1).broadcast_to([xl, 2, Y])
                    nc.vector.tensor_mul(dst, src2, rr)
                else:
                    nc.vector.tensor_mul(
                        st[0:xl, ci0 * Y : (ci0 + 1) * Y],
                        eep[0:xl, 0:Y],
                        recips[j][0:xl, :],
                    )
                if (unit[-1] % CG == CG - 1) or unit[-1] == CC - 1:
                    # one DMA per (group, stripe), alternating HWDGE engines
                    dram = out[g * CG : g * CG + gc, xo : xo + xl, :].rearrange(
                        "c x y -> x c y"
                    )
                    eng = nc.sync if (j + g) % 2 == 0 else nc.scalar
                    eng.dma_start(dram, st[0:xl, 0 : gc * Y])

        # software pipeline: emit stage1(u+1) before stage2(u) so the PE
        # works through the next unit while ACT drains T1 psum of this one.
        t1s = [stage1(c) for c in units[0]]
        for i, unit in enumerate(units):
            t1s_next = (
                [stage1(c) for c in units[i + 1]] if i + 1 < len(units) else None
            )
            emit_stage2(unit, t1s)
            t1s = t1s_next

    nc.compile()
    return nc


def kernel(wt, x_in_lon, x_in_lat, x_out_lon, x_out_lat, init_ls):
    global LAST_RESULT
    wt = np.asarray(wt, dtype=np.float32)
    x_in_lon = np.asarray(x_in_lon, dtype=np.float32)
    x_in_lat = np.asarray(x_in_lat, dtype=np.float32)
    x_out_lon = np.asarray(x_out_lon, dtype=np.float32)
    x_out_lat = np.asarray(x_out_lat, dtype=np.float32)
    ls = float(np.asarray(init_ls).reshape(-1)[0])
    alpha = -0.5 / (ls * ls)

    # density channel + nan cleanup, then [B, CC, W, H] -> [B, W, CC*H]
    density = (~np.isnan(wt[:, 0:1])).astype(np.float32)
    wt_aug = np.concatenate([density, np.nan_to_num(wt, nan=0.0)], axis=1)
    wtr = np.ascontiguousarray(wt_aug.transpose(0, 2, 1, 3)).reshape(B, W, CC * H)
    if MM_DTYPE == "bf16":
        import ml_dtypes

        wtr = wtr.astype(ml_dtypes.bfloat16)

    key = (alpha, MM_DTYPE)
    if key not in _cache:
        _cache[key] = _build(alpha, MM_DTYPE)
    nc = _cache[key]

    in_maps = [
        {
            "wtr": wtr[b],
            "lon_in": x_in_lon[b : b + 1],
            "lon_out": x_out_lon[b : b + 1],
            "lat_in": x_in_lat[b : b + 1],
            "lat_out": x_out_lat[b : b + 1],
        }
        for b in range(B)
    ]
    res = run_bass_kernel_spmd(nc, in_maps, list(range(B)), trace=TRACE)
    LAST_RESULT = res
    return np.stack([res.results[b]["out"] for b in range(B)])



# revision 5
# speedup vs baseline: 1.2302x; 1.2302x over previous
"""ConvDeepSet SPMD kernel for 8 Trainium2 NeuronCores.

Math (per batch b, all fp32 in reference):
    density = 1 where wt[:,0] finite else 0            [1,W,H]
    wt_aug  = concat([density, nan_to_num(wt)])        [CC=33,W,H]
    w0[w,x] = exp(-0.5*(lon_in[w]-lon_out[x])^2/ls^2)  [W,X]
    w1[h,y] = exp(-0.5*(lat_in[h]-lat_out[y])^2/ls^2)  [H,Y]
    ee[c,x,y] = sum_{w,h} wt_aug[c,w,h]*w0[w,x]*w1[h,y]
    out[0]   = ee[0];  out[c>=1] = ee[c] / clip(ee[0], 1e-6, 1e5)

Sharding: data-parallel over batch B=8 -> one NeuronCore per batch.

v2 design notes (driven by the v1 ntff profile):
  * v1 was DMA packet-rate bound: fp32 output (34.3 MB/core) written as
    [CC,X,Y] gave 1444-byte DRAM runs -> 23.7k packets on 16 DMA engines
    (~100 ns each) = 132 us DMA busy.  v2 writes bf16 to a [X, CC*Y]
    DRAM layout: per-partition runs of 5.8-6.5 KB (full 2 KB packets),
    ~9.5k packets, and the host undoes the transpose for free.
  * normalize muls (psum fp32 -> sbuf bf16) run at 1 elem/cycle on DVE;
    69k elem/partition would be 72 us on DVE alone, so stripes are split
    DVE(4) / GpSimd(2) per channel pair.
  * stage-1 psum -> sbuf copies are one strided ACT op per channel
    (halves at psum cols 0/512 so each matmul stays in one bank).
  * output DMA fires per (channel-group, stripe) so writes overlap
    compute; triggers alternate between the Sync and ACT HWDGE queues.
"""

import sys
from contextlib import ExitStack

import numpy as np

sys.path.insert(0, "/opt/trn_rl_repo")

import concourse.bass as bass  # noqa: E402,F401
import concourse.tile as tile  # noqa: E402
from concourse import bacc, mybir  # noqa: E402
from concourse.bass_utils import run_bass_kernel_spmd  # noqa: E402

B, C, W, H, X, Y = 8, 32, 256, 128, 720, 361
CC = C + 1          # channels incl. density
KT = W // 128       # stage-1 K tiles (2)
N1 = 360            # stage-1 half width (psum halves at cols 0 / 512)
XOFF = [0, 128, 256, 384, 512, 640]   # stage-2 x stripes (5x128 + 80)
XLEN = [128, 128, 128, 128, 128, 80]
NXT = len(XOFF)
# output DMA channel groups: contiguous [c0, c1) ranges of the CC dim
OGROUPS = [(0, 8), (8, 16), (16, 24), (24, 33)]
GP_STRIPES = (4, 5)   # stripes normalized on GpSimd; rest on DVE

F32 = mybir.dt.float32
BF16 = mybir.dt.bfloat16

MM_DTYPE = "bf16"
TRACE = False
LAST_RESULT = None

_cache = {}


def _build(alpha: float, mm: str):
    nc = bacc.Bacc(
        "TRN2",
        target_bir_lowering=False,
        debug=False,
        enable_asserts=False,
        num_devices=B,
    )
    mmdt = {"f32": F32, "f32r": mybir.dt.float32r, "bf16": BF16}[mm]

    wtr = nc.dram_tensor("wtr", [W, CC * H], mmdt, kind="ExternalInput").ap()
    lon_in = nc.dram_tensor("lon_in", [1, W], F32, kind="ExternalInput").ap()
    lon_out = nc.dram_tensor("lon_out", [1, X], F32, kind="ExternalInput").ap()
    lat_in = nc.dram_tensor("lat_in", [1, H], F32, kind="ExternalInput").ap()
    lat_out = nc.dram_tensor("lat_out", [1, Y], F32, kind="ExternalInput").ap()
    # output laid out [x, c, y] so each partition line is one long
    # contiguous DRAM run; host transposes back to [c, x, y].
    out = nc.dram_tensor("out", [X, CC * Y], BF16, kind="ExternalOutput").ap()

    with tile.TileContext(nc) as tc, ExitStack() as ctx:
        wtr_pool = ctx.enter_context(tc.tile_pool(name="wtr", bufs=KT))
        w0_pool = ctx.enter_context(tc.tile_pool(name="w0", bufs=KT))
        w1_pool = ctx.enter_context(tc.tile_pool(name="w1", bufs=1))
        t1sb_pool = ctx.enter_context(tc.tile_pool(name="t1sb", bufs=6))
        recip_pool = ctx.enter_context(tc.tile_pool(name="recip", bufs=NXT))
        rs_pool = ctx.enter_context(tc.tile_pool(name="rs", bufs=2))
        stage_pool = ctx.enter_context(tc.tile_pool(name="stage", bufs=NXT))
        small_pool = ctx.enter_context(tc.tile_pool(name="small", bufs=2))
        t1ps_pool = ctx.enter_context(tc.tile_pool(name="t1ps", bufs=2, space="PSUM"))
        eeps_pool = ctx.enter_context(tc.tile_pool(name="eeps", bufs=2, space="PSUM"))

        wtr_sb = [
            wtr_pool.tile([128, CC * H], mmdt, tag="wtr", name=f"wtr_sb{k}")
            for k in range(KT)
        ]

        # ---- RBF weights: w[p, x] = exp(alpha * (a_p - b_x)^2) on ACT,
        # with b broadcast across partitions and a as per-partition bias.
        def rbf(in_ap, out_ap, n_in, n_out, w_sb):
            bb = small_pool.tile([128, n_out], F32, tag="rbf_bb", name=f"rbf_bb{n_out}")
            nc.sync.dma_start(bb[:], out_ap.to_broadcast([128, n_out]))
            for k in range(n_in // 128):
                ar = small_pool.tile(
                    [128, 1], F32, tag="rbf_ar", name=f"rbf_ar{n_in}_{k}"
                )
                nc.sync.dma_start(
                    ar[:],
                    in_ap[0:1, k * 128 : (k + 1) * 128].rearrange("a b -> b a"),
                )
                d2 = small_pool.tile(
                    [128, n_out], F32, tag="rbf_d2", name=f"rbf_d2{n_in}_{k}"
                )
                nc.scalar.activation(
                    d2[:],
                    bb[:],
                    mybir.ActivationFunctionType.Square,
                    bias=ar[:],
                    scale=-1.0,
                )
                nc.scalar.activation(
                    w_sb[k][:],
                    d2[:],
                    mybir.ActivationFunctionType.Exp,
                    scale=alpha,
                )

        w0_sb = [
            w0_pool.tile([128, X], mmdt, tag="w0", name=f"w0_sb{k}")
            for k in range(KT)
        ]
        rbf(lon_in, lon_out, W, X, w0_sb)
        w1_sb = w1_pool.tile([128, Y], mmdt, tag="w1", name="w1_sb0")
        rbf(lat_in, lat_out, H, Y, [w1_sb])

        # ---- load wt (stage-1 stationary operand) in 4 chunk DMAs so the
        # first stage-1 matmuls start after ~1/4 of the load.
        chunks = [(0, 16), (16, 33)]
        for ci, (a, b) in enumerate(chunks):
            for k in range(KT):
                eng = nc.sync if (ci + k) % 2 == 0 else nc.scalar
                eng.dma_start(
                    wtr_sb[k][:, a * H : b * H],
                    wtr[k * 128 : (k + 1) * 128, a * H : b * H],
                )

        # per-stripe staging tiles holding ALL channels: [x_part, c*Y + y]
        stage_tiles = [
            stage_pool.tile([128, CC * Y], BF16, tag="stage", name=f"stage_{j}")
            for j in range(NXT)
        ]
        recips = [
            recip_pool.tile([128, Y], F32, tag="recip", name=f"recip{j}")
            for j in range(NXT)
        ]

        # ---- stage 1 for one channel: T1[h, x] in one 2-bank psum tile
        # (halves at cols 0 / 512), one strided ACT copy -> sbuf bf16.
        def stage1(c):
            t1sb = t1sb_pool.tile([128, X], mmdt, tag="t1sb", name=f"t1sb_c{c}")
            t1ps = t1ps_pool.tile([128, 1024], F32, tag="t1ps", name=f"t1ps_c{c}")
            for n in range(2):
                for k in range(KT):
                    nc.tensor.matmul(
                        t1ps[:, n * 512 : n * 512 + N1],
                        wtr_sb[k][:, c * H : (c + 1) * H],
                        w0_sb[k][:, n * N1 : (n + 1) * N1],
                        start=(k == 0),
                        stop=(k == KT - 1),
                    )
            src = t1ps[:].rearrange("p (b y) -> p b y", b=2)[:, :, 0:N1]
            dst = t1sb[:].rearrange("p (b y) -> p b y", b=2)
            nc.scalar.copy(dst, src)
            return t1sb

        units = [[0]] + [[2 * i + 1, 2 * i + 2] for i in range(16)]

        def emit_stage2(unit, t1sbs):
            c0 = unit[0]
            for j in range(NXT):
                xo, xl = XOFF[j], XLEN[j]
                eep = eeps_pool.tile(
                    [128, 1024], F32, tag="ee", name=f"ee_u{c0}_{j}"
                )
                for idx in range(len(unit)):
                    nc.tensor.matmul(
                        eep[0:xl, idx * 512 : idx * 512 + Y],
                        t1sbs[idx][:, xo : xo + xl],
                        w1_sb[:],
                        start=True,
                        stop=True,
                    )
                st = stage_tiles[j]
                if c0 == 0:
                    # density channel: copy out (ACT) + clipped reciprocal (DVE)
                    nc.scalar.copy(st[0:xl, 0:Y], eep[0:xl, 0:Y])
                    rs = rs_pool.tile([128, Y], F32, tag="rs", name=f"rs{j}")
                    nc.vector.tensor_scalar(
                        rs[0:xl, :],
                        eep[0:xl, 0:Y],
                        1e-6,
                        1e5,
                        mybir.AluOpType.max,
                        mybir.AluOpType.min,
                    )
                    nc.vector.reciprocal_approx_fast(
                        recips[j][0:xl, :], rs[0:xl, :]
                    )
                elif j in GP_STRIPES:
                    # GpSimd can't read PSUM: ACT stages the raw bf16 copy
                    # (strided pair copy), then GpSimd scales in-place in SBUF.
                    src2 = eep[0:xl, :].rearrange("p (b y) -> p b y", b=2)[
                        :, :, 0:Y
                    ]
                    dstp = st[0:xl, c0 * Y : (c0 + 2) * Y].rearrange(
                        "p (b y) -> p b y", b=2
                    )
                    nc.scalar.copy(dstp, src2)
                    for idx in range(len(unit)):
                        ci = unit[idx]
                        nc.gpsimd.tensor_mul(
                            st[0:xl, ci * Y : (ci + 1) * Y],
                            st[0:xl, ci * Y : (ci + 1) * Y],
                            recips[j][0:xl, :],
                        )
                else:
                    # DVE: one strided op scaling both channels of the pair
                    src2 = eep[0:xl, :].rearrange("p (b y) -> p b y", b=2)[
                        :, :, 0:Y
                    ]
                    dst = st[0:xl, c0 * Y : (c0 + 2) * Y].rearrange(
                        "p (b y) -> p b y", b=2
                    )
                    rr = recips[j][0:xl, :].unsqueeze(1).broadcast_to([xl, 2, Y])
                    nc.vector.tensor_mul(dst, src2, rr)
                # output DMA once the last channel of a group is staged
                for g, (ga, gb) in enumerate(OGROUPS):
                    if unit[-1] == gb - 1 or (
                        len(unit) == 2 and unit[0] == gb - 1
                    ):
                        nc.sync.dma_start(
                            out[xo : xo + xl, ga * Y : gb * Y],
                            st[0:xl, ga * Y : gb * Y],
                        )

        # software pipeline: emit stage1(u+1) before stage2(u) so the PE
        # works through the next unit while ACT drains T1 psum of this one.
        t1s = [stage1(c) for c in units[0]]
        for i, unit in enumerate(units):
            t1s_next = (
                [stage1(c) for c in units[i + 1]] if i + 1 < len(units) else None
            )
            emit_stage2(unit, t1s)
            t1s = t1s_next

    nc.compile()
    return nc


def kernel(wt, x_in_lon, x_in_lat, x_out_lon, x_out_lat, init_ls):
    global LAST_RESULT
    wt = np.asarray(wt, dtype=np.float32)
    x_in_lon = np.asarray(x_in_lon, dtype=np.float32)
    x_in_lat = np.asarray(x_in_lat, dtype=np.float32)
    x_out_lon = np.asarray(x_out_lon, dtype=np.float32)
    x_out_lat = np.asarray(x_out_lat, dtype=np.float32)
    ls = float(np.asarray(init_ls).reshape(-1)[0])
    alpha = -0.5 / (ls * ls)

    # density channel + nan cleanup, then [B, CC, W, H] -> [B, W, CC*H]
    density = (~np.isnan(wt[:, 0:1])).astype(np.float32)
    wt_aug = np.concatenate([density, np.nan_to_num(wt, nan=0.0)], axis=1)
    wtr = np.ascontiguousarray(wt_aug.transpose(0, 2, 1, 3)).reshape(B, W, CC * H)
    if MM_DTYPE == "bf16":
        import ml_dtypes

        wtr = wtr.astype(ml_dtypes.bfloat16)

    key = (alpha, MM_DTYPE)
    if key not in _cache:
        _cache[key] = _build(alpha, MM_DTYPE)
    nc = _cache[key]

    in_maps = [
        {
            "wtr": wtr[b],
            "lon_in": x_in_lon[b : b + 1],
            "lon_out": x_out_lon[b : b + 1],
            "lat_in": x_in_lat[b : b + 1],
            "lat_out": x_out_lat[b : b + 1],
        }
        for b in range(B)
    ]
    res = run_bass_kernel_spmd(nc, in_maps, list(range(B)), trace=TRACE)
    LAST_RESULT = res
    # device wrote [X, CC*Y] bf16; reassemble to [B, CC, X, Y] fp32
    outs = []
    for b in range(B):
        o = np.asarray(res.results[b]["out"]).astype(np.float32)
        outs.append(o.reshape(X, CC, Y).transpose(1, 0, 2))
    return np.stack(outs)


# revision 9
# speedup vs baseline: 1.4623x; 1.1887x over previous
"""ConvDeepSet SPMD kernel for 8 Trainium2 NeuronCores.

Math (per batch b, all fp32 in reference):
    density = 1 where wt[:,0] finite else 0            [1,W,H]
    wt_aug  = concat([density, nan_to_num(wt)])        [CC=33,W,H]
    w0[w,x] = exp(-0.5*(lon_in[w]-lon_out[x])^2/ls^2)  [W,X]
    w1[h,y] = exp(-0.5*(lat_in[h]-lat_out[y])^2/ls^2)  [H,Y]
    ee[c,x,y] = sum_{w,h} wt_aug[c,w,h]*w0[w,x]*w1[h,y]
    out[0]   = ee[0];  out[c>=1] = ee[c] / clip(ee[0], 1e-6, 1e5)

Sharding: data-parallel over batch B=8 -> one NeuronCore per batch.

v2 design notes (driven by the v1 ntff profile):
  * v1 was DMA packet-rate bound: fp32 output (34.3 MB/core) written as
    [CC,X,Y] gave 1444-byte DRAM runs -> 23.7k packets on 16 DMA engines
    (~100 ns each) = 132 us DMA busy.  v2 writes bf16 to a [X, CC*Y]
    DRAM layout: per-partition runs of 5.8-6.5 KB (full 2 KB packets),
    ~9.5k packets, and the host undoes the transpose for free.
  * normalize muls (psum fp32 -> sbuf bf16) run at 1 elem/cycle on DVE;
    69k elem/partition would be 72 us on DVE alone, so stripes are split
    DVE(4) / GpSimd(2) per channel pair.
  * stage-1 psum -> sbuf copies are one strided ACT op per channel
    (halves at psum cols 0/512 so each matmul stays in one bank).
  * output DMA fires per (channel-group, stripe) so writes overlap
    compute; triggers alternate between the Sync and ACT HWDGE queues.
"""

import sys
from contextlib import ExitStack

import numpy as np

sys.path.insert(0, "/opt/trn_rl_repo")

import concourse.bass as bass  # noqa: E402,F401
import concourse.tile as tile  # noqa: E402
from concourse import bacc, mybir  # noqa: E402
from concourse.bass_utils import run_bass_kernel_spmd  # noqa: E402

B, C, W, H, X, Y = 8, 32, 256, 128, 720, 361
CC = C + 1          # channels incl. density
KT = W // 128       # stage-1 K tiles (2)
N1 = 360            # stage-1 half width (psum halves at cols 0 / 512)
XOFF = [0, 128, 256, 384, 512, 640]   # stage-2 x stripes (5x128 + 80)
XLEN = [128, 128, 128, 128, 128, 80]
NXT = len(XOFF)
# output DMA channel groups: contiguous [c0, c1) ranges of the CC dim
OGROUPS = [(0, 8), (8, 16), (16, 24), (24, 33)]
GP_STRIPES = (0, 1)   # stripes normalized on GpSimd; rest on DVE

F32 = mybir.dt.float32
BF16 = mybir.dt.bfloat16

MM_DTYPE = "bf16"
TRACE = False
LAST_RESULT = None

_cache = {}


def _build(alpha: float, mm: str):
    nc = bacc.Bacc(
        "TRN2",
        target_bir_lowering=False,
        debug=False,
        enable_asserts=False,
        num_devices=B,
    )
    mmdt = {"f32": F32, "f32r": mybir.dt.float32r, "bf16": BF16}[mm]

    wtr = nc.dram_tensor("wtr", [W, CC * H], mmdt, kind="ExternalInput").ap()
    lon_in = nc.dram_tensor("lon_in", [1, W], F32, kind="ExternalInput").ap()
    lon_out = nc.dram_tensor("lon_out", [1, X], F32, kind="ExternalInput").ap()
    lat_in = nc.dram_tensor("lat_in", [1, H], F32, kind="ExternalInput").ap()
    lat_out = nc.dram_tensor("lat_out", [1, Y], F32, kind="ExternalInput").ap()
    # output laid out [x, c, y] so each partition line is one long
    # contiguous DRAM run; host transposes back to [c, x, y].
    out = nc.dram_tensor("out", [X, CC * Y], BF16, kind="ExternalOutput").ap()

    with tile.TileContext(nc) as tc, ExitStack() as ctx:
        wtr_pool = ctx.enter_context(tc.tile_pool(name="wtr", bufs=KT))
        w0_pool = ctx.enter_context(tc.tile_pool(name="w0", bufs=KT))
        w1_pool = ctx.enter_context(tc.tile_pool(name="w1", bufs=1))
        t1sb_pool = ctx.enter_context(tc.tile_pool(name="t1sb", bufs=6))
        recip_pool = ctx.enter_context(tc.tile_pool(name="recip", bufs=NXT))
        rs_pool = ctx.enter_context(tc.tile_pool(name="rs", bufs=2))
        stage_pool = ctx.enter_context(tc.tile_pool(name="stage", bufs=NXT))
        small_pool = ctx.enter_context(tc.tile_pool(name="small", bufs=2))
        t1ps_pool = ctx.enter_context(tc.tile_pool(name="t1ps", bufs=2, space="PSUM"))
        eeps_pool = ctx.enter_context(tc.tile_pool(name="eeps", bufs=3, space="PSUM"))

        wtr_sb = [
            wtr_pool.tile([128, CC * H], mmdt, tag="wtr", name=f"wtr_sb{k}")
            for k in range(KT)
        ]

        # ---- RBF weights: w[p, x] = exp(alpha * (a_p - b_x)^2) on ACT,
        # with b broadcast across partitions and a as per-partition bias.
        def rbf(in_ap, out_ap, n_in, n_out, w_sb):
            bb = small_pool.tile([128, n_out], F32, tag="rbf_bb", name=f"rbf_bb{n_out}")
            nc.sync.dma_start(bb[:], out_ap.to_broadcast([128, n_out]))
            for k in range(n_in // 128):
                ar = small_pool.tile(
                    [128, 1], F32, tag="rbf_ar", name=f"rbf_ar{n_in}_{k}"
                )
                nc.sync.dma_start(
                    ar[:],
                    in_ap[0:1, k * 128 : (k + 1) * 128].rearrange("a b -> b a"),
                )
                d2 = small_pool.tile(
                    [128, n_out], F32, tag="rbf_d2", name=f"rbf_d2{n_in}_{k}"
                )
                nc.scalar.activation(
                    d2[:],
                    bb[:],
                    mybir.ActivationFunctionType.Square,
                    bias=ar[:],
                    scale=-1.0,
                )
                nc.scalar.activation(
                    w_sb[k][:],
                    d2[:],
                    mybir.ActivationFunctionType.Exp,
                    scale=alpha,
                )

        w0_sb = [
            w0_pool.tile([128, X], mmdt, tag="w0", name=f"w0_sb{k}")
            for k in range(KT)
        ]
        rbf(lon_in, lon_out, W, X, w0_sb)
        w1_sb = w1_pool.tile([128, Y], mmdt, tag="w1", name="w1_sb0")
        rbf(lat_in, lat_out, H, Y, [w1_sb])

        # ---- load wt (stage-1 stationary operand) in 4 chunk DMAs so the
        # first stage-1 matmuls start after ~1/4 of the load.
        chunks = [(0, 16), (16, 33)]
        for ci, (a, b) in enumerate(chunks):
            for k in range(KT):
                eng = nc.sync if (ci + k) % 2 == 0 else nc.scalar
                eng.dma_start(
                    wtr_sb[k][:, a * H : b * H],
                    wtr[k * 128 : (k + 1) * 128, a * H : b * H],
                )

        # per-stripe staging tiles holding ALL channels: [x_part, c*Y + y]
        stage_tiles = [
            stage_pool.tile([128, CC * Y], BF16, tag="stage", name=f"stage_{j}")
            for j in range(NXT)
        ]
        recips = [
            recip_pool.tile([128, Y], F32, tag="recip", name=f"recip{j}")
            for j in range(NXT)
        ]

        # ---- stage 1 for one channel: T1[h, x] in two 1-bank psum halves,
        # each ACT-copied (converting to bf16) into t1sb as soon as ready.
        def stage1(c):
            t1sb = t1sb_pool.tile([128, X], mmdt, tag="t1sb", name=f"t1sb_c{c}")
            for n in range(2):
                t1ps = t1ps_pool.tile(
                    [128, N1], F32, tag="t1ps", name=f"t1ps_c{c}_{n}"
                )
                for k in range(KT):
                    nc.tensor.matmul(
                        t1ps[:],
                        wtr_sb[k][:, c * H : (c + 1) * H],
                        w0_sb[k][:, n * N1 : (n + 1) * N1],
                        start=(k == 0),
                        stop=(k == KT - 1),
                    )
                nc.scalar.copy(t1sb[:, n * N1 : (n + 1) * N1], t1ps[:])
            return t1sb

        units = [[0]] + [[2 * i + 1, 2 * i + 2] for i in range(16)]

        def emit_stage2(unit, t1sbs):
            c0 = unit[0]
            for j in range(NXT):
                xo, xl = XOFF[j], XLEN[j]
                eep = eeps_pool.tile(
                    [128, 1024], F32, tag="ee", name=f"ee_u{c0}_{j}"
                )
                for idx in range(len(unit)):
                    nc.tensor.matmul(
                        eep[0:xl, idx * 512 : idx * 512 + Y],
                        t1sbs[idx][:, xo : xo + xl],
                        w1_sb[:],
                        start=True,
                        stop=True,
                    )
                st = stage_tiles[j]
                if c0 == 0:
                    # density channel: copy out (ACT) + clipped reciprocal (DVE)
                    nc.scalar.copy(st[0:xl, 0:Y], eep[0:xl, 0:Y])
                    rs = rs_pool.tile([128, Y], F32, tag="rs", name=f"rs{j}")
                    nc.vector.tensor_scalar(
                        rs[0:xl, :],
                        eep[0:xl, 0:Y],
                        1e-6,
                        1e5,
                        mybir.AluOpType.max,
                        mybir.AluOpType.min,
                    )
                    nc.vector.reciprocal_approx_fast(
                        recips[j][0:xl, :], rs[0:xl, :]
                    )
                elif j in GP_STRIPES:
                    # GpSimd can't read PSUM: ACT stages the raw bf16 copy
                    # (strided pair copy), then GpSimd scales in-place in SBUF.
                    src2 = eep[0:xl, :].rearrange("p (b y) -> p b y", b=2)[
                        :, :, 0:Y
                    ]
                    dstp = st[0:xl, c0 * Y : (c0 + 2) * Y].rearrange(
                        "p (b y) -> p b y", b=2
                    )
                    nc.scalar.copy(dstp, src2)
                    for idx in range(len(unit)):
                        ci = unit[idx]
                        nc.gpsimd.tensor_mul(
                            st[0:xl, ci * Y : (ci + 1) * Y],
                            st[0:xl, ci * Y : (ci + 1) * Y],
                            recips[j][0:xl, :],
                        )
                else:
                    # DVE: one strided op scaling both channels of the pair
                    src2 = eep[0:xl, :].rearrange("p (b y) -> p b y", b=2)[
                        :, :, 0:Y
                    ]
                    dst = st[0:xl, c0 * Y : (c0 + 2) * Y].rearrange(
                        "p (b y) -> p b y", b=2
                    )
                    rr = recips[j][0:xl, :].unsqueeze(1).broadcast_to([xl, 2, Y])
                    nc.vector.tensor_mul(dst, src2, rr)
                # output DMA once the last channel of a group is staged
                for g, (ga, gb) in enumerate(OGROUPS):
                    if unit[-1] == gb - 1 or (
                        len(unit) == 2 and unit[0] == gb - 1
                    ):
                        nc.sync.dma_start(
                            out[xo : xo + xl, ga * Y : gb * Y],
                            st[0:xl, ga * Y : gb * Y],
                        )

        # software pipeline, lookahead 1: t1sb(u) is ready one iteration
        # early; stage2(u) is emitted BEFORE stage1(u+1) so the ACT queue
        # runs the psum-freeing gp-stripe copies of pair u ahead of the
        # next pair's t1 copies (eeps bufs=2 reuse would stall PE otherwise).
        t1s = [stage1(c) for c in units[0]]
        t1s = [t1s, [stage1(c) for c in units[1]]]
        for i, unit in enumerate(units):
            emit_stage2(unit, t1s[0])
            t1s = t1s[1:]
            if i + 2 < len(units):
                t1s.append([stage1(c) for c in units[i + 2]])

    nc.compile()
    return nc


def kernel(wt, x_in_lon, x_in_lat, x_out_lon, x_out_lat, init_ls):
    global LAST_RESULT
    wt = np.asarray(wt, dtype=np.float32)
    x_in_lon = np.asarray(x_in_lon, dtype=np.float32)
    x_in_lat = np.asarray(x_in_lat, dtype=np.float32)
    x_out_lon = np.asarray(x_out_lon, dtype=np.float32)
    x_out_lat = np.asarray(x_out_lat, dtype=np.float32)
    ls = float(np.asarray(init_ls).reshape(-1)[0])
    alpha = -0.5 / (ls * ls)

    # density channel + nan cleanup, then [B, CC, W, H] -> [B, W, CC*H]
    density = (~np.isnan(wt[:, 0:1])).astype(np.float32)
    wt_aug = np.concatenate([density, np.nan_to_num(wt, nan=0.0)], axis=1)
    wtr = np.ascontiguousarray(wt_aug.transpose(0, 2, 1, 3)).reshape(B, W, CC * H)
    if MM_DTYPE == "bf16":
        import ml_dtypes

        wtr = wtr.astype(ml_dtypes.bfloat16)

    key = (alpha, MM_DTYPE)
    if key not in _cache:
        _cache[key] = _build(alpha, MM_DTYPE)
    nc = _cache[key]

    in_maps = [
        {
            "wtr": wtr[b],
            "lon_in": x_in_lon[b : b + 1],
            "lon_out": x_out_lon[b : b + 1],
            "lat_in": x_in_lat[b : b + 1],
            "lat_out": x_out_lat[b : b + 1],
        }
        for b in range(B)
    ]
    res = run_bass_kernel_spmd(nc, in_maps, list(range(B)), trace=TRACE)
    LAST_RESULT = res
    # device wrote [X, CC*Y] bf16; reassemble to [B, CC, X, Y] fp32
    outs = []
    for b in range(B):
        o = np.asarray(res.results[b]["out"]).astype(np.float32)
        outs.append(o.reshape(X, CC, Y).transpose(1, 0, 2))
    return np.stack(outs)


# revision 13
# speedup vs baseline: 1.5540x; 1.0627x over previous
"""ConvDeepSet SPMD kernel for 8 Trainium2 NeuronCores.

Math (per batch b, all fp32 in reference):
    density = 1 where wt[:,0] finite else 0            [1,W,H]
    wt_aug  = concat([density, nan_to_num(wt)])        [CC=33,W,H]
    w0[w,x] = exp(-0.5*(lon_in[w]-lon_out[x])^2/ls^2)  [W,X]
    w1[h,y] = exp(-0.5*(lat_in[h]-lat_out[y])^2/ls^2)  [H,Y]
    ee[c,x,y] = sum_{w,h} wt_aug[c,w,h]*w0[w,x]*w1[h,y]
    out[0]   = ee[0];  out[c>=1] = ee[c] / clip(ee[0], 1e-6, 1e5)

Sharding: data-parallel over batch B=8 -> one NeuronCore per batch.

v2 design notes (driven by the v1 ntff profile):
  * v1 was DMA packet-rate bound: fp32 output (34.3 MB/core) written as
    [CC,X,Y] gave 1444-byte DRAM runs -> 23.7k packets on 16 DMA engines
    (~100 ns each) = 132 us DMA busy.  v2 writes bf16 to a [X, CC*Y]
    DRAM layout: per-partition runs of 5.8-6.5 KB (full 2 KB packets),
    ~9.5k packets, and the host undoes the transpose for free.
  * normalize muls (psum fp32 -> sbuf bf16) run at 1 elem/cycle on DVE;
    69k elem/partition would be 72 us on DVE alone, so stripes are split
    DVE(4) / GpSimd(2) per channel pair.
  * stage-1 psum -> sbuf copies are one strided ACT op per channel
    (halves at psum cols 0/512 so each matmul stays in one bank).
  * output DMA fires per (channel-group, stripe) so writes overlap
    compute; triggers alternate between the Sync and ACT HWDGE queues.
"""

import sys
from contextlib import ExitStack

import numpy as np

sys.path.insert(0, "/opt/trn_rl_repo")

import concourse.bass as bass  # noqa: E402,F401
import concourse.tile as tile  # noqa: E402
from concourse import bacc, mybir  # noqa: E402
from concourse.bass_utils import run_bass_kernel_spmd  # noqa: E402

B, C, W, H, X, Y = 8, 32, 256, 128, 720, 361
CC = C + 1          # channels incl. density
KT = W // 128       # stage-1 K tiles (2)
N1 = 360            # stage-1 half width (psum halves at cols 0 / 512)
XOFF = [0, 128, 256, 384, 512, 640]   # stage-2 x stripes (5x128 + 80)
XLEN = [128, 128, 128, 128, 128, 80]
NXT = len(XOFF)
# output DMA channel groups: contiguous [c0, c1) ranges of the CC dim.
# Trailing groups are smaller so the final DMAs (which cannot overlap
# anything) move less data.
OGROUPS = [(0, 8), (8, 16), (16, 24), (24, 31), (31, 33)]
GP_STRIPES = (0, 1)   # stripes normalized on GpSimd; rest on DVE

F32 = mybir.dt.float32
BF16 = mybir.dt.bfloat16

MM_DTYPE = "bf16"
TRACE = False
LAST_RESULT = None

_cache = {}


def _build(alpha: float, mm: str):
    nc = bacc.Bacc(
        "TRN2",
        target_bir_lowering=False,
        debug=False,
        enable_asserts=False,
        num_devices=B,
    )
    mmdt = {"f32": F32, "f32r": mybir.dt.float32r, "bf16": BF16}[mm]

    wtr = nc.dram_tensor("wtr", [W, CC * H], mmdt, kind="ExternalInput").ap()
    lon_in = nc.dram_tensor("lon_in", [1, W], F32, kind="ExternalInput").ap()
    lon_out = nc.dram_tensor("lon_out", [1, X], F32, kind="ExternalInput").ap()
    lat_in = nc.dram_tensor("lat_in", [1, H], F32, kind="ExternalInput").ap()
    lat_out = nc.dram_tensor("lat_out", [1, Y], F32, kind="ExternalInput").ap()
    # output laid out [x, c, y] so each partition line is one long
    # contiguous DRAM run; host transposes back to [c, x, y].
    out = nc.dram_tensor("out", [X, CC * Y], BF16, kind="ExternalOutput").ap()

    with tile.TileContext(nc) as tc, ExitStack() as ctx:
        wtr_pool = ctx.enter_context(tc.tile_pool(name="wtr", bufs=KT))
        w0_pool = ctx.enter_context(tc.tile_pool(name="w0", bufs=KT))
        w1_pool = ctx.enter_context(tc.tile_pool(name="w1", bufs=1))
        t1sb_pool = ctx.enter_context(tc.tile_pool(name="t1sb", bufs=6))
        recip_pool = ctx.enter_context(tc.tile_pool(name="recip", bufs=NXT))
        rs_pool = ctx.enter_context(tc.tile_pool(name="rs", bufs=2))
        stage_pool = ctx.enter_context(tc.tile_pool(name="stage", bufs=NXT))
        small_pool = ctx.enter_context(tc.tile_pool(name="small", bufs=2))
        t1ps_pool = ctx.enter_context(tc.tile_pool(name="t1ps", bufs=2, space="PSUM"))
        eeps_pool = ctx.enter_context(tc.tile_pool(name="eeps", bufs=3, space="PSUM"))

        wtr_sb = [
            wtr_pool.tile([128, CC * H], mmdt, tag="wtr", name=f"wtr_sb{k}")
            for k in range(KT)
        ]

        # ---- RBF weights: w[p, x] = exp(alpha * (a_p - b_x)^2) on ACT,
        # with b broadcast across partitions and a as per-partition bias.
        # `a` reaches the partition dim via a cheap PE transpose (a strided
        # [n,1] DMA would cost n tiny descriptors and stall the pipeline head).
        ident = small_pool.tile([1, 1], F32, tag="ident", name="ident")
        nc.vector.memset(ident[:], 1.0)

        def rbf(in_ap, out_ap, n_in, n_out, w_sb):
            bb = small_pool.tile([128, n_out], F32, tag="rbf_bb", name=f"rbf_bb{n_out}")
            nc.sync.dma_start(bb[:], out_ap.to_broadcast([128, n_out]))
            row = small_pool.tile([1, n_in], F32, tag="rbf_row", name=f"rbf_row{n_in}")
            nc.sync.dma_start(row[:], in_ap[0:1, :])
            for k in range(n_in // 128):
                arp = t1ps_pool.tile([128, 8], F32, tag="t1ps", name=f"arp{n_in}_{k}")
                nc.tensor.transpose(
                    arp[:, 0:1], row[0:1, k * 128 : (k + 1) * 128], ident[:]
                )
                ar = small_pool.tile(
                    [128, 1], F32, tag="rbf_ar", name=f"rbf_ar{n_in}_{k}"
                )
                nc.scalar.copy(ar[:], arp[:, 0:1])
                d2 = small_pool.tile(
                    [128, n_out], F32, tag="rbf_d2", name=f"rbf_d2{n_in}_{k}"
                )
                nc.scalar.activation(
                    d2[:],
                    bb[:],
                    mybir.ActivationFunctionType.Square,
                    bias=ar[:],
                    scale=-1.0,
                )
                nc.scalar.activation(
                    w_sb[k][:],
                    d2[:],
                    mybir.ActivationFunctionType.Exp,
                    scale=alpha,
                )

        w0_sb = [
            w0_pool.tile([128, X], mmdt, tag="w0", name=f"w0_sb{k}")
            for k in range(KT)
        ]
        rbf(lon_in, lon_out, W, X, w0_sb)
        w1_sb = w1_pool.tile([128, Y], mmdt, tag="w1", name="w1_sb0")
        rbf(lat_in, lat_out, H, Y, [w1_sb])

        # ---- load wt (stage-1 stationary operand) in 4 chunk DMAs so the
        # first stage-1 matmuls start after ~1/4 of the load.
        chunks = [(0, 16), (16, 33)]
        for ci, (a, b) in enumerate(chunks):
            for k in range(KT):
                nc.sync.dma_start(
                    wtr_sb[k][:, a * H : b * H],
                    wtr[k * 128 : (k + 1) * 128, a * H : b * H],
                )

        # per-stripe staging tiles holding ALL channels: [x_part, c*Y + y]
        stage_tiles = [
            stage_pool.tile([128, CC * Y], BF16, tag="stage", name=f"stage_{j}")
            for j in range(NXT)
        ]
        recips = [
            recip_pool.tile([128, Y], F32, tag="recip", name=f"recip{j}")
            for j in range(NXT)
        ]

        # ---- stage 1 for one channel: T1[h, x] in two 1-bank psum halves,
        # each ACT-copied (converting to bf16) into t1sb as soon as ready.
        def stage1(c):
            t1sb = t1sb_pool.tile([128, X], mmdt, tag="t1sb", name=f"t1sb_c{c}")
            for n in range(2):
                t1ps = t1ps_pool.tile(
                    [128, N1], F32, tag="t1ps", name=f"t1ps_c{c}_{n}"
                )
                for k in range(KT):
                    nc.tensor.matmul(
                        t1ps[:],
                        wtr_sb[k][:, c * H : (c + 1) * H],
                        w0_sb[k][:, n * N1 : (n + 1) * N1],
                        start=(k == 0),
                        stop=(k == KT - 1),
                    )
                nc.scalar.copy(t1sb[:, n * N1 : (n + 1) * N1], t1ps[:])
            return t1sb

        units = [[0]] + [[2 * i + 1, 2 * i + 2] for i in range(16)]

        def emit_stage2(unit, t1sbs):
            c0 = unit[0]
            for j in range(NXT):
                xo, xl = XOFF[j], XLEN[j]
                eep = eeps_pool.tile(
                    [128, 1024], F32, tag="ee", name=f"ee_u{c0}_{j}"
                )
                for idx in range(len(unit)):
                    nc.tensor.matmul(
                        eep[0:xl, idx * 512 : idx * 512 + Y],
                        t1sbs[idx][:, xo : xo + xl],
                        w1_sb[:],
                        start=True,
                        stop=True,
                    )
                st = stage_tiles[j]
                if c0 == 0:
                    # density channel: copy out (ACT) + clipped reciprocal (DVE)
                    nc.scalar.copy(st[0:xl, 0:Y], eep[0:xl, 0:Y])
                    rs = rs_pool.tile([128, Y], F32, tag="rs", name=f"rs{j}")
                    nc.vector.tensor_scalar(
                        rs[0:xl, :],
                        eep[0:xl, 0:Y],
                        1e-6,
                        1e5,
                        mybir.AluOpType.max,
                        mybir.AluOpType.min,
                    )
                    nc.vector.reciprocal_approx_fast(
                        recips[j][0:xl, :], rs[0:xl, :]
                    )
                elif j in GP_STRIPES:
                    # GpSimd can't read PSUM: ACT stages the raw bf16 copy
                    # (strided pair copy), then GpSimd scales in-place in SBUF.
                    src2 = eep[0:xl, :].rearrange("p (b y) -> p b y", b=2)[
                        :, :, 0:Y
                    ]
                    dstp = st[0:xl, c0 * Y : (c0 + 2) * Y].rearrange(
                        "p (b y) -> p b y", b=2
                    )
                    nc.scalar.copy(dstp, src2)
                    for idx in range(len(unit)):
                        ci = unit[idx]
                        nc.gpsimd.tensor_mul(
                            st[0:xl, ci * Y : (ci + 1) * Y],
                            st[0:xl, ci * Y : (ci + 1) * Y],
                            recips[j][0:xl, :],
                        )
                else:
                    # DVE: one strided op scaling both channels of the pair
                    src2 = eep[0:xl, :].rearrange("p (b y) -> p b y", b=2)[
                        :, :, 0:Y
                    ]
                    dst = st[0:xl, c0 * Y : (c0 + 2) * Y].rearrange(
                        "p (b y) -> p b y", b=2
                    )
                    rr = recips[j][0:xl, :].unsqueeze(1).broadcast_to([xl, 2, Y])
                    nc.vector.tensor_mul(dst, src2, rr)
                # output DMA once the last channel of a group is staged
                for g, (ga, gb) in enumerate(OGROUPS):
                    if unit[-1] == gb - 1 or (
                        len(unit) == 2 and unit[0] == gb - 1
                    ):
                        # GP-normalized stripes trigger from the GP queue
                        # (zero wait: GP's own mul precedes it); the rest
                        # from Sync so waits don't block GP's compute.
                        eng = nc.gpsimd if j in GP_STRIPES else nc.sync
                        eng.dma_start(
                            out[xo : xo + xl, ga * Y : gb * Y],
                            st[0:xl, ga * Y : gb * Y],
                        )

        # software pipeline, lookahead 1: t1sb(u) is ready one iteration
        # early; stage2(u) is emitted BEFORE stage1(u+1) so the ACT queue
        # runs the psum-freeing gp-stripe copies of pair u ahead of the
        # next pair's t1 copies (eeps bufs=2 reuse would stall PE otherwise).
        t1s = [stage1(c) for c in units[0]]
        t1s = [t1s, [stage1(c) for c in units[1]]]
        for i, unit in enumerate(units):
            emit_stage2(unit, t1s[0])
            t1s = t1s[1:]
            if i + 2 < len(units):
                t1s.append([stage1(c) for c in units[i + 2]])

    nc.compile()
    return nc


def kernel(wt, x_in_lon, x_in_lat, x_out_lon, x_out_lat, init_ls):
    global LAST_RESULT
    wt = np.asarray(wt, dtype=np.float32)
    x_in_lon = np.asarray(x_in_lon, dtype=np.float32)
    x_in_lat = np.asarray(x_in_lat, dtype=np.float32)
    x_out_lon = np.asarray(x_out_lon, dtype=np.float32)
    x_out_lat = np.asarray(x_out_lat, dtype=np.float32)
    ls = float(np.asarray(init_ls).reshape(-1)[0])
    alpha = -0.5 / (ls * ls)

    # density channel + nan cleanup, then [B, CC, W, H] -> [B, W, CC*H]
    density = (~np.isnan(wt[:, 0:1])).astype(np.float32)
    wt_aug = np.concatenate([density, np.nan_to_num(wt, nan=0.0)], axis=1)
    wtr = np.ascontiguousarray(wt_aug.transpose(0, 2, 1, 3)).reshape(B, W, CC * H)
    if MM_DTYPE == "bf16":
        import ml_dtypes

        wtr = wtr.astype(ml_dtypes.bfloat16)

    key = (alpha, MM_DTYPE)
    if key not in _cache:
        _cache[key] = _build(alpha, MM_DTYPE)
    nc = _cache[key]

    in_maps = [
        {
            "wtr": wtr[b],
            "lon_in": x_in_lon[b : b + 1],
            "lon_out": x_out_lon[b : b + 1],
            "lat_in": x_in_lat[b : b + 1],
            "lat_out": x_out_lat[b : b + 1],
        }
        for b in range(B)
    ]
    res = run_bass_kernel_spmd(nc, in_maps, list(range(B)), trace=TRACE)
    LAST_RESULT = res
    # device wrote [X, CC*Y] bf16; reassemble to [B, CC, X, Y] fp32
    outs = []
    for b in range(B):
        o = np.asarray(res.results[b]["out"]).astype(np.float32)
        outs.append(o.reshape(X, CC, Y).transpose(1, 0, 2))
    return np.stack(outs)


# revision 16
# speedup vs baseline: 1.6561x; 1.0657x over previous
"""ConvDeepSet SPMD kernel for 8 Trainium2 NeuronCores.

Math (per batch b, all fp32 in reference):
    density = 1 where wt[:,0] finite else 0            [1,W,H]
    wt_aug  = concat([density, nan_to_num(wt)])        [CC=33,W,H]
    w0[w,x] = exp(-0.5*(lon_in[w]-lon_out[x])^2/ls^2)  [W,X]
    w1[h,y] = exp(-0.5*(lat_in[h]-lat_out[y])^2/ls^2)  [H,Y]
    ee[c,x,y] = sum_{w,h} wt_aug[c,w,h]*w0[w,x]*w1[h,y]
    out[0]   = ee[0];  out[c>=1] = ee[c] / clip(ee[0], 1e-6, 1e5)

Sharding: data-parallel over batch B=8 -> one NeuronCore per batch.

v2 design notes (driven by the v1 ntff profile):
  * v1 was DMA packet-rate bound: fp32 output (34.3 MB/core) written as
    [CC,X,Y] gave 1444-byte DRAM runs -> 23.7k packets on 16 DMA engines
    (~100 ns each) = 132 us DMA busy.  v2 writes bf16 to a [X, CC*Y]
    DRAM layout: per-partition runs of 5.8-6.5 KB (full 2 KB packets),
    ~9.5k packets, and the host undoes the transpose for free.
  * normalize muls (psum fp32 -> sbuf bf16) run at 1 elem/cycle on DVE;
    69k elem/partition would be 72 us on DVE alone, so stripes are split
    DVE(4) / GpSimd(2) per channel pair.
  * stage-1 psum -> sbuf copies are one strided ACT op per channel
    (halves at psum cols 0/512 so each matmul stays in one bank).
  * output DMA fires per (channel-group, stripe) so writes overlap
    compute; triggers alternate between the Sync and ACT HWDGE queues.
"""

import sys
from contextlib import ExitStack

import numpy as np

sys.path.insert(0, "/opt/trn_rl_repo")

import concourse.bass as bass  # noqa: E402,F401
import concourse.tile as tile  # noqa: E402
from concourse import bacc, mybir  # noqa: E402
from concourse.bass_utils import run_bass_kernel_spmd  # noqa: E402

B, C, W, H, X, Y = 8, 32, 256, 128, 720, 361
CC = C + 1          # channels incl. density
KT = W // 128       # stage-1 K tiles (2)
N1 = 360            # stage-1 half width (psum halves at cols 0 / 512)
XOFF = [0, 128, 256, 384, 512, 640]   # stage-2 x stripes (5x128 + 80)
XLEN = [128, 128, 128, 128, 128, 80]
NXT = len(XOFF)
# output DMA channel groups: contiguous [c0, c1) ranges of the CC dim.
# GP stripes (triggered from the GpSimd queue) use coarse groups to keep
# that queue light; DVE stripes fire fine-grained from Sync every ~2 pairs
# so output DMA overlaps compute smoothly. Trailing groups are smaller so
# the final, non-overlappable DMAs move less data.
OGROUPS = [(0, 8), (8, 16), (16, 24), (24, 31), (31, 33)]
AGROUPS = [
    (0, 1), (1, 5), (5, 9), (9, 13), (13, 17), (17, 21),
    (21, 25), (25, 29), (29, 31), (31, 33),
]
GP_STRIPES = (0, 1)   # stripes normalized on GpSimd; rest on DVE

F32 = mybir.dt.float32
BF16 = mybir.dt.bfloat16

MM_DTYPE = "bf16"
TRACE = False
LAST_RESULT = None

_cache = {}


def _build(alpha: float, mm: str):
    nc = bacc.Bacc(
        "TRN2",
        target_bir_lowering=False,
        debug=False,
        enable_asserts=False,
        num_devices=B,
    )
    mmdt = {"f32": F32, "f32r": mybir.dt.float32r, "bf16": BF16}[mm]

    wtr = nc.dram_tensor("wtr", [W, CC * H], mmdt, kind="ExternalInput").ap()
    lon_in = nc.dram_tensor("lon_in", [1, W], F32, kind="ExternalInput").ap()
    lon_out = nc.dram_tensor("lon_out", [1, X], F32, kind="ExternalInput").ap()
    lat_in = nc.dram_tensor("lat_in", [1, H], F32, kind="ExternalInput").ap()
    lat_out = nc.dram_tensor("lat_out", [1, Y], F32, kind="ExternalInput").ap()
    # output laid out [x, c, y] so each partition line is one long
    # contiguous DRAM run; host transposes back to [c, x, y].
    out = nc.dram_tensor("out", [X, CC * Y], BF16, kind="ExternalOutput").ap()

    with tile.TileContext(nc) as tc, ExitStack() as ctx:
        wtr_pool = ctx.enter_context(tc.tile_pool(name="wtr", bufs=KT))
        w0_pool = ctx.enter_context(tc.tile_pool(name="w0", bufs=KT))
        w1_pool = ctx.enter_context(tc.tile_pool(name="w1", bufs=1))
        t1sb_pool = ctx.enter_context(tc.tile_pool(name="t1sb", bufs=6))
        recip_pool = ctx.enter_context(tc.tile_pool(name="recip", bufs=NXT))
        rs_pool = ctx.enter_context(tc.tile_pool(name="rs", bufs=2))
        stage_pool = ctx.enter_context(tc.tile_pool(name="stage", bufs=NXT))
        small_pool = ctx.enter_context(tc.tile_pool(name="small", bufs=2))
        t1ps_pool = ctx.enter_context(tc.tile_pool(name="t1ps", bufs=2, space="PSUM"))
        eeps_pool = ctx.enter_context(tc.tile_pool(name="eeps", bufs=3, space="PSUM"))

        wtr_sb = [
            wtr_pool.tile([128, CC * H], mmdt, tag="wtr", name=f"wtr_sb{k}")
            for k in range(KT)
        ]

        # ---- RBF weights: w[p, x] = exp(alpha * (a_p - b_x)^2) on ACT,
        # with b broadcast across partitions and a as per-partition bias.
        # `a` reaches the partition dim via a cheap PE transpose (a strided
        # [n,1] DMA would cost n tiny descriptors and stall the pipeline head).
        ident = small_pool.tile([1, 1], F32, tag="ident", name="ident")
        nc.vector.memset(ident[:], 1.0)

        def rbf(in_ap, out_ap, n_in, n_out, w_sb):
            bb = small_pool.tile([128, n_out], F32, tag="rbf_bb", name=f"rbf_bb{n_out}")
            nc.sync.dma_start(bb[:], out_ap.to_broadcast([128, n_out]))
            row = small_pool.tile([1, n_in], F32, tag="rbf_row", name=f"rbf_row{n_in}")
            nc.sync.dma_start(row[:], in_ap[0:1, :])
            for k in range(n_in // 128):
                arp = t1ps_pool.tile([128, 8], F32, tag="t1ps", name=f"arp{n_in}_{k}")
                nc.tensor.transpose(
                    arp[:, 0:1], row[0:1, k * 128 : (k + 1) * 128], ident[:]
                )
                ar = small_pool.tile(
                    [128, 1], F32, tag="rbf_ar", name=f"rbf_ar{n_in}_{k}"
                )
                nc.scalar.copy(ar[:], arp[:, 0:1])
                d2 = small_pool.tile(
                    [128, n_out], F32, tag="rbf_d2", name=f"rbf_d2{n_in}_{k}"
                )
                nc.scalar.activation(
                    d2[:],
                    bb[:],
                    mybir.ActivationFunctionType.Square,
                    bias=ar[:],
                    scale=-1.0,
                )
                nc.scalar.activation(
                    w_sb[k][:],
                    d2[:],
                    mybir.ActivationFunctionType.Exp,
                    scale=alpha,
                )

        w0_sb = [
            w0_pool.tile([128, X], mmdt, tag="w0", name=f"w0_sb{k}")
            for k in range(KT)
        ]
        rbf(lon_in, lon_out, W, X, w0_sb)
        w1_sb = w1_pool.tile([128, Y], mmdt, tag="w1", name="w1_sb0")
        rbf(lat_in, lat_out, H, Y, [w1_sb])

        # ---- load wt (stage-1 stationary operand) in 4 chunk DMAs so the
        # first stage-1 matmuls start after ~1/4 of the load.
        chunks = [(0, 4), (4, 16), (16, 33)]
        for ci, (a, b) in enumerate(chunks):
            for k in range(KT):
                nc.sync.dma_start(
                    wtr_sb[k][:, a * H : b * H],
                    wtr[k * 128 : (k + 1) * 128, a * H : b * H],
                )

        # per-stripe staging tiles holding ALL channels: [x_part, c*Y + y]
        stage_tiles = [
            stage_pool.tile([128, CC * Y], BF16, tag="stage", name=f"stage_{j}")
            for j in range(NXT)
        ]
        recips = [
            recip_pool.tile([128, Y], F32, tag="recip", name=f"recip{j}")
            for j in range(NXT)
        ]

        # ---- stage 1 for one channel: T1[h, x] in two 1-bank psum halves,
        # each ACT-copied (converting to bf16) into t1sb as soon as ready.
        def stage1(c):
            t1sb = t1sb_pool.tile([128, X], mmdt, tag="t1sb", name=f"t1sb_c{c}")
            for n in range(2):
                t1ps = t1ps_pool.tile(
                    [128, N1], F32, tag="t1ps", name=f"t1ps_c{c}_{n}"
                )
                for k in range(KT):
                    nc.tensor.matmul(
                        t1ps[:],
                        wtr_sb[k][:, c * H : (c + 1) * H],
                        w0_sb[k][:, n * N1 : (n + 1) * N1],
                        start=(k == 0),
                        stop=(k == KT - 1),
                    )
                nc.scalar.copy(t1sb[:, n * N1 : (n + 1) * N1], t1ps[:])
            return t1sb

        units = [[0]] + [[2 * i + 1, 2 * i + 2] for i in range(16)]

        def emit_stage2(unit, t1sbs):
            c0 = unit[0]
            for j in range(NXT):
                xo, xl = XOFF[j], XLEN[j]
                eep = eeps_pool.tile(
                    [128, 1024], F32, tag="ee", name=f"ee_u{c0}_{j}"
                )
                for idx in range(len(unit)):
                    nc.tensor.matmul(
                        eep[0:xl, idx * 512 : idx * 512 + Y],
                        t1sbs[idx][:, xo : xo + xl],
                        w1_sb[:],
                        start=True,
                        stop=True,
                    )
                st = stage_tiles[j]
                if c0 == 0:
                    # density channel: copy out (ACT) + clipped reciprocal (DVE)
                    nc.scalar.copy(st[0:xl, 0:Y], eep[0:xl, 0:Y])
                    rs = rs_pool.tile([128, Y], F32, tag="rs", name=f"rs{j}")
                    nc.vector.tensor_scalar(
                        rs[0:xl, :],
                        eep[0:xl, 0:Y],
                        1e-6,
                        1e5,
                        mybir.AluOpType.max,
                        mybir.AluOpType.min,
                    )
                    nc.vector.reciprocal_approx_fast(
                        recips[j][0:xl, :], rs[0:xl, :]
                    )
                elif j in GP_STRIPES:
                    # GpSimd can't read PSUM: ACT stages the raw bf16 copy
                    # (strided pair copy), then GpSimd scales in-place in SBUF.
                    src2 = eep[0:xl, :].rearrange("p (b y) -> p b y", b=2)[
                        :, :, 0:Y
                    ]
                    dstp = st[0:xl, c0 * Y : (c0 + 2) * Y].rearrange(
                        "p (b y) -> p b y", b=2
                    )
                    nc.scalar.copy(dstp, src2)
                    for idx in range(len(unit)):
                        ci = unit[idx]
                        nc.gpsimd.tensor_mul(
                            st[0:xl, ci * Y : (ci + 1) * Y],
                            st[0:xl, ci * Y : (ci + 1) * Y],
                            recips[j][0:xl, :],
                        )
                else:
                    # DVE: one strided op scaling both channels of the pair
                    src2 = eep[0:xl, :].rearrange("p (b y) -> p b y", b=2)[
                        :, :, 0:Y
                    ]
                    dst = st[0:xl, c0 * Y : (c0 + 2) * Y].rearrange(
                        "p (b y) -> p b y", b=2
                    )
                    rr = recips[j][0:xl, :].unsqueeze(1).broadcast_to([xl, 2, Y])
                    nc.vector.tensor_mul(dst, src2, rr)
                # output DMA once the last channel of a group is staged
                groups = OGROUPS if j in GP_STRIPES else AGROUPS
                for ga, gb in groups:
                    if unit[-1] == gb - 1 or (
                        len(unit) == 2 and unit[0] == gb - 1
                    ):
                        # GP-normalized stripes trigger from the GP queue
                        # (zero wait: GP's own mul precedes it); the rest
                        # from Sync so waits don't block GP's compute.
                        eng = nc.gpsimd if j in GP_STRIPES else nc.sync
                        eng.dma_start(
                            out[xo : xo + xl, ga * Y : gb * Y],
                            st[0:xl, ga * Y : gb * Y],
                        )

        # software pipeline, lookahead 1: t1sb(u) is ready one iteration
        # early; stage2(u) is emitted BEFORE stage1(u+1) so the ACT queue
        # runs the psum-freeing gp-stripe copies of pair u ahead of the
        # next pair's t1 copies (eeps bufs=2 reuse would stall PE otherwise).
        t1s = [stage1(c) for c in units[0]]
        t1s = [t1s, [stage1(c) for c in units[1]]]
        for i, unit in enumerate(units):
            emit_stage2(unit, t1s[0])
            t1s = t1s[1:]
            if i + 2 < len(units):
                t1s.append([stage1(c) for c in units[i + 2]])

    nc.compile()
    return nc


def kernel(wt, x_in_lon, x_in_lat, x_out_lon, x_out_lat, init_ls):
    global LAST_RESULT
    wt = np.asarray(wt, dtype=np.float32)
    x_in_lon = np.asarray(x_in_lon, dtype=np.float32)
    x_in_lat = np.asarray(x_in_lat, dtype=np.float32)
    x_out_lon = np.asarray(x_out_lon, dtype=np.float32)
    x_out_lat = np.asarray(x_out_lat, dtype=np.float32)
    ls = float(np.asarray(init_ls).reshape(-1)[0])
    alpha = -0.5 / (ls * ls)

    # density channel + nan cleanup, then [B, CC, W, H] -> [B, W, CC*H]
    density = (~np.isnan(wt[:, 0:1])).astype(np.float32)
    wt_aug = np.concatenate([density, np.nan_to_num(wt, nan=0.0)], axis=1)
    wtr = np.ascontiguousarray(wt_aug.transpose(0, 2, 1, 3)).reshape(B, W, CC * H)
    if MM_DTYPE == "bf16":
        import ml_dtypes

        wtr = wtr.astype(ml_dtypes.bfloat16)

    key = (alpha, MM_DTYPE)
    if key not in _cache:
        _cache[key] = _build(alpha, MM_DTYPE)
    nc = _cache[key]

    in_maps = [
        {
            "wtr": wtr[b],
            "lon_in": x_in_lon[b : b + 1],
            "lon_out": x_out_lon[b : b + 1],
            "lat_in": x_in_lat[b : b + 1],
            "lat_out": x_out_lat[b : b + 1],
        }
        for b in range(B)
    ]
    res = run_bass_kernel_spmd(nc, in_maps, list(range(B)), trace=TRACE)
    LAST_RESULT = res
    # device wrote [X, CC*Y] bf16; reassemble to [B, CC, X, Y] fp32
    outs = []
    for b in range(B):
        o = np.asarray(res.results[b]["out"]).astype(np.float32)
        outs.append(o.reshape(X, CC, Y).transpose(1, 0, 2))
    return np.stack(outs)
